# revision 18
# baseline (speedup 1.0000x reference)
"""Trainium2 Bass kernel for nn_FEDformer_69750268887102.

Strategy: data-parallel over batch across 8 NeuronCores (4 sequences/core).
Residual stream kept resident in SBUF in channel-major layout [128, 8, 2048]
fp32. FFN / embedding / layernorm matmuls run as float32r (full PE speed at
free-dim >= 256, fp32 storage). The whole Fourier-attention branch runs in
bf16: its weights are scaled by 1/D^2 so the branch contributes ~1e-5
absolute to the residual.

rfft + mode-select + irfft are reformulated as small dense matmuls
(64 selected bins -> [512,128] DFT and [128,512] iDFT bases built on host).
The FEDformer view(B,L,-1) reshape quirk on the [B,H,E,L] tensor is realized
as a zero-cost strided access-pattern view.
"""

import numpy as np
import ml_dtypes

import concourse.bass as bass
import concourse.mybir as mybir
import concourse.tile as tile
from concourse import bacc
from concourse.bass_utils import run_bass_kernel_spmd
from concourse.masks import make_identity

# dims
B, L, C = 32, 512, 7
D, H, DFF, NL, MODES, NCLS = 1024, 8, 4096, 4, 64, 2
E, MA, P = 128, 25, 128
NCORES = 8
BL = B // NCORES          # 4 batches per core
T = BL * L                # 2048 tokens per core
KC = D // P               # 8 d-chunks
FC = DFF // P             # 32 dff-chunks

F32 = mybir.dt.float32
F32R = mybir.dt.float32r
BF16 = mybir.dt.bfloat16
AX = mybir.AxisListType
OP = mybir.AluOpType
AF = mybir.ActivationFunctionType
bfnp = ml_dtypes.bfloat16


# ---------------------------------------------------------------- host prep

def _host_prep(inputs):
    x_enc = np.asarray(inputs["x_enc"], np.float32)
    token_w = np.asarray(inputs["token_w"], np.float32)
    qw = np.asarray(inputs["qw"], np.float32)
    qb = np.asarray(inputs["qb"], np.float32)
    ow = np.asarray(inputs["ow"], np.float32)
    ob = np.asarray(inputs["ob"], np.float32)
    wfr = np.asarray(inputs["wfr"], np.float32)
    wfi = np.asarray(inputs["wfi"], np.float32)
    c1w = np.asarray(inputs["c1w"], np.float32)
    c2w = np.asarray(inputs["c2w"], np.float32)
    lnw = np.asarray(inputs["lnw"], np.float32)
    lnb = np.asarray(inputs["lnb"], np.float32)
    proj_w = np.asarray(inputs["proj_w"], np.float32)
    proj_b = np.asarray(inputs["proj_b"], np.float32)
    modes_index = np.asarray(inputs["modes_index"])

    # embedding im2col (circular conv k=3): xcol[b, c*3+k, l] = x_enc.T[b,c,(l+k-1)%L]
    xt = x_enc.transpose(0, 2, 1)                                    # [B, C, L]
    idx = (np.arange(L)[None, :] + np.arange(3)[:, None] - 1) % L    # [3, L]
    xcol = xt[:, :, idx].reshape(B, C * 3, L)                        # [B, 21, L]
    xcol_p = np.zeros((B, 32, L), np.float32)
    xcol_p[:, : C * 3] = xcol
    # W2[(c*3+k), d] = token_w[d, c, k], padded to 32 rows
    w2 = np.zeros((32, D), np.float32)
    w2[: C * 3] = token_w.transpose(1, 2, 0).reshape(C * 3, D)

    # positional embedding, channel-major [D, L]
    pos = np.arange(L, dtype=np.float32)[:, None]
    div = np.exp(np.arange(0, D, 2, dtype=np.float32) * (-np.log(10000.0) / D))
    pe = np.zeros((L, D), np.float32)
    pe[:, 0::2] = np.sin(pos * div)
    pe[:, 1::2] = np.cos(pos * div)
    peT = np.ascontiguousarray(pe.T)                                 # [D, L]

    # DFT basis [L, 128]: cols 0:64 Re (cos), 64:128 Im (-sin) at selected bins
    ll = np.arange(L)
    kk = modes_index.astype(np.int64)
    ee = np.exp(-2j * np.pi * np.outer(ll, kk) / L)
    ftm = np.concatenate([ee.real, ee.imag], axis=1).astype(np.float32)  # [512,128]

    # iDFT basis [128, L]: exact linear map of irfft with bins 0..63 populated
    imp = np.zeros((MODES, L // 2 + 1), np.complex64)
    imp[np.arange(MODES), np.arange(MODES)] = 1.0
    cr = np.fft.irfft(imp, n=L, axis=-1)
    impi = np.zeros((MODES, L // 2 + 1), np.complex64)
    impi[np.arange(MODES), np.arange(MODES)] = 1j
    ci = np.fft.irfft(impi, n=L, axis=-1)
    cmat = np.concatenate([cr, ci], axis=0).astype(np.float32)       # [128, 512]

    # weights, pre-transposed/arranged for the device layouts
    qwT = np.ascontiguousarray(qw.transpose(0, 2, 1)).astype(bfnp)   # [NL, D(in), D(out)]
    owT = np.ascontiguousarray(ow.transpose(0, 2, 1)).astype(bfnp)   # [NL, D(d'), D(n)]
    # Fourier weights arranged [NL, H, E_i, M, E_o] so (h, mode-slab) DMAs are contiguous
    wrA = np.ascontiguousarray(wfr.transpose(0, 1, 2, 4, 3)).astype(bfnp)
    wiA = np.ascontiguousarray(wfi.transpose(0, 1, 2, 4, 3)).astype(bfnp)
    # c1w [NL, DFF, D] -> pre[i, fc, p, c, n] = c1w[i, fc*128+n, c*128+p]
    c1pre = np.ascontiguousarray(
        c1w.reshape(NL, FC, 128, KC, 128).transpose(0, 1, 4, 3, 2)
    )  # [NL, FC, 128, KC, 128] f32
    # c2w [NL, D, DFF]: need c2wT[dff, dout] natural layout
    c2T = np.ascontiguousarray(c2w.transpose(0, 2, 1))               # [NL, DFF, D] f32
    # classification head channel-major: pw[n, l*D+d] -> [n, d, l]
    pwcm = np.ascontiguousarray(
        proj_w.reshape(NCLS, L, D).transpose(0, 2, 1)
    )  # [NCLS, D, L] f32
    pbt = np.tile(proj_b, BL).astype(np.float32)[None, :]            # [1, 2*BL]

    shared = {
        "w2": w2,
        "pe": peT,
        "ftm": ftm.astype(bfnp),
        "cmat": cmat.astype(bfnp),
        "qwt": qwT,
        "owt": owT,
        "qb": qb.astype(bfnp),
        "ob": ob,
        "wra": wrA,
        "wia": wiA,
        "c1p": c1pre,
        "c2t": c2T,
        "lnw": lnw,
        "lnb": lnb,
        "pwc": pwcm,
        "pbt": pbt,
        "onesr": np.ones((128, 128), np.float32),
    }
    per_core = []
    for c in range(NCORES):
        sl = xcol_p[c * BL:(c + 1) * BL]                             # [BL, 32, L]
        xc = np.ascontiguousarray(sl.transpose(1, 0, 2).reshape(32, T))
        m = dict(shared)
        m["xcol"] = xc
        per_core.append(m)
    return per_core


# ---------------------------------------------------------------- bass build

def build_nc():
    nc = bacc.Bacc("TRN2", target_bir_lowering=False, debug=False)

    d_xcol = nc.dram_tensor("xcol", [32, T], F32R, kind="ExternalInput").ap()
    d_w2 = nc.dram_tensor("w2", [32, D], F32R, kind="ExternalInput").ap()
    d_pe = nc.dram_tensor("pe", [D, L], F32, kind="ExternalInput").ap()
    d_ft = nc.dram_tensor("ftm", [L, 2 * MODES], BF16, kind="ExternalInput").ap()
    d_cm = nc.dram_tensor("cmat", [2 * MODES, L], BF16, kind="ExternalInput").ap()
    d_qw = nc.dram_tensor("qwt", [NL, D, D], BF16, kind="ExternalInput").ap()
    d_ow = nc.dram_tensor("owt", [NL, D, D], BF16, kind="ExternalInput").ap()
    d_qb = nc.dram_tensor("qb", [NL, D], BF16, kind="ExternalInput").ap()
    d_ob = nc.dram_tensor("ob", [NL, D], F32, kind="ExternalInput").ap()
    d_wr = nc.dram_tensor("wra", [NL, H, E, MODES, E], BF16, kind="ExternalInput").ap()
    d_wi = nc.dram_tensor("wia", [NL, H, E, MODES, E], BF16, kind="ExternalInput").ap()
    d_c1 = nc.dram_tensor("c1p", [NL, FC, 128, KC, 128], F32R, kind="ExternalInput").ap()
    d_c2 = nc.dram_tensor("c2t", [NL, DFF, D], F32R, kind="ExternalInput").ap()
    d_lnw = nc.dram_tensor("lnw", [D], F32, kind="ExternalInput").ap()
    d_lnb = nc.dram_tensor("lnb", [D], F32, kind="ExternalInput").ap()
    d_pw = nc.dram_tensor("pwc", [NCLS, D, L], F32, kind="ExternalInput").ap()
    d_pb = nc.dram_tensor("pbt", [1, NCLS * BL], F32, kind="ExternalInput").ap()
    d_ones = nc.dram_tensor("onesr", [128, 128], F32R, kind="ExternalInput").ap()
    d_out = nc.dram_tensor("out", [1, NCLS * BL], F32, kind="ExternalOutput").ap()

    with tile.TileContext(nc) as tc:
        _emit(nc, tc, d_xcol, d_w2, d_pe, d_ft, d_cm, d_qw, d_ow, d_qb, d_ob,
              d_wr, d_wi, d_c1, d_c2, d_lnw, d_lnb, d_pw, d_pb, d_out, d_ones)
    nc.compile()
    return nc


def _emit(nc, tc, d_xcol, d_w2, d_pe, d_ft, d_cm, d_qw, d_ow, d_qb, d_ob,
          d_wr, d_wi, d_c1, d_c2, d_lnw, d_lnb, d_pw, d_pb, d_out, d_ones):
    from contextlib import ExitStack

    def f32r(ap):
        return ap.bitcast(F32R)

    with ExitStack() as top:
        pres = top.enter_context(tc.tile_pool(name="pres", bufs=1))
        pconst = top.enter_context(tc.tile_pool(name="pconst", bufs=1))

        # resident residual stream, channel-major [p, dc, (b l)]
        xsb = pres.tile([128, KC, T], F32R)

        ft_sb = pconst.tile([128, 4, 2 * MODES], BF16)      # FT chunks [l'c]
        nc.sync.dma_start(ft_sb, d_ft.rearrange("(c p) m -> p c m", p=128))
        cm_sb = pconst.tile([128, L], BF16)                 # Cmat [mc, l]
        nc.sync.dma_start(cm_sb, d_cm)
        ident = pconst.tile([128, 128], BF16)
        make_identity(nc, ident)
        ones_sb = pconst.tile([128, 128], F32R)
        nc.sync.dma_start(ones_sb, d_ones)
        ones_bf = pconst.tile([1, 128], BF16)
        nc.vector.memset(ones_bf, 1.0)
        eps_sb = pconst.tile([128, 1], F32)
        nc.vector.memset(eps_sb, 1e-5)
        ones32 = pconst.tile([128, 1], F32)
        nc.vector.memset(ones32, 1.0)
        ln_sb = pconst.tile([128, 2 * KC], F32)             # lnw | lnb
        nc.sync.dma_start(ln_sb[:, :KC], d_lnw.rearrange("(c p) -> p c", p=128))
        nc.sync.dma_start(ln_sb[:, KC:], d_lnb.rearrange("(c p) -> p c", p=128))

        # ---------------- embedding ----------------
        with ExitStack() as st:
            pemb = st.enter_context(tc.tile_pool(name="pemb", bufs=1))
            ppe_ = st.enter_context(tc.tile_pool(name="ppemb", bufs=2, space="PSUM"))
            xcol_sb = pemb.tile([32, T], F32R)
            nc.sync.dma_start(xcol_sb, d_xcol)
            w2_sb = pemb.tile([32, D], F32R)
            nc.sync.dma_start(w2_sb, d_w2)
            pe_sb = pemb.tile([128, KC, L], F32)
            nc.sync.dma_start(pe_sb, d_pe.rearrange("(c p) l -> p c l", p=128))
            for dc in range(KC):
                for b in range(BL):
                    ps = ppe_.tile([128, 512], F32)
                    nc.tensor.matmul(
                        ps,
                        w2_sb[:, dc * 128:(dc + 1) * 128],
                        xcol_sb[:, b * 512:(b + 1) * 512],
                        start=True, stop=True,
                    )
                    nc.vector.tensor_tensor(
                        xsb[:, dc, b * 512:(b + 1) * 512], ps, pe_sb[:, dc], OP.add
                    )

        # ---------------- encoder layers ----------------
        for i in range(NL):
            _emit_attn(nc, tc, i, xsb, ft_sb, cm_sb, ident, ones_bf,
                       d_qw, d_ow, d_qb, d_ob, d_wr, d_wi)
            _emit_decomp(nc, tc, f"d{i}a", xsb)
            _emit_ffn(nc, tc, i, xsb, d_c1, d_c2)
            _emit_decomp(nc, tc, f"d{i}b", xsb)

        # ---------------- final layernorm + head ----------------
        with ExitStack() as st:
            pf = st.enter_context(tc.tile_pool(name="pfin", bufs=2))
            ppf = st.enter_context(tc.tile_pool(name="ppfin", bufs=2, space="PSUM"))
            ppw = st.enter_context(tc.tile_pool(name="ppw", bufs=1))

            for b in range(BL):
                sl = slice(b * 512, (b + 1) * 512)
                mu_ps = ppf.tile([128, 512], F32, tag="mu")
                s2_ps = ppf.tile([128, 512], F32, tag="s2")
                for dc in range(KC):
                    nc.tensor.matmul(mu_ps, ones_sb, xsb[:, dc, sl],
                                     start=(dc == 0), stop=(dc == KC - 1))
                for dc in range(KC):
                    sq_t = pf.tile([128, 512], F32R, tag="sq")
                    nc.scalar.activation(sq_t, xsb[:, dc, sl], AF.Square)
                    nc.tensor.matmul(s2_ps, ones_sb, sq_t,
                                     start=(dc == 0), stop=(dc == KC - 1))
                # mu, var, rstd as full [128, 512] broadcast tiles
                mu_t = pf.tile([128, 512], F32, tag="mut")
                nc.vector.tensor_scalar_mul(mu_t, mu_ps, 1.0 / D)
                m2_t = pf.tile([128, 512], F32, tag="m2t")
                nc.vector.tensor_tensor(m2_t, mu_t, mu_t, OP.mult)
                var_t = pf.tile([128, 512], F32, tag="vart")
                nc.vector.scalar_tensor_tensor(var_t, s2_ps, 1.0 / D, m2_t,
                                               OP.mult, OP.subtract)
                sd_t = pf.tile([128, 512], F32, tag="sdt")
                nc.scalar.activation(sd_t, var_t, AF.Sqrt, bias=eps_sb)
                rs_t = pf.tile([128, 512], F32, tag="rst")
                nc.vector.reciprocal(rs_t, sd_t)
                for dc in range(KC):
                    xv = xsb[:, dc, sl]
                    nc.vector.tensor_tensor(xv, xv, mu_t, OP.subtract)
                    nc.vector.tensor_tensor(xv, xv, rs_t, OP.mult)
                    nc.vector.tensor_scalar(
                        xv, xv, ln_sb[:, dc:dc + 1], ln_sb[:, KC + dc:KC + dc + 1],
                        OP.mult, OP.add,
                    )
                    # subtract per-(channel, batch) time mean
                    r_t = pf.tile([128, 1], F32, tag="rt")
                    nc.vector.tensor_reduce(r_t, xv, AX.X, OP.add)
                    nc.vector.tensor_scalar_mul(r_t, r_t, 1.0 / L)
                    nc.vector.tensor_scalar_sub(xv, xv, r_t)
                    nc.scalar.activation(xv, xv, AF.Gelu)

            pw_sb = ppw.tile([128, NCLS, KC, 512], F32)
            nc.sync.dma_start(pw_sb, d_pw.rearrange("n (c p) l -> p n c l", p=128))
            pb_sb = ppw.tile([1, NCLS * BL], F32)
            nc.sync.dma_start(pb_sb, d_pb)
            ob_out = ppw.tile([1, NCLS * BL], F32)
            for b in range(BL):
                sl = slice(b * 512, (b + 1) * 512)
                for n in range(NCLS):
                    acc = pf.tile([128, 512], F32, tag="acc")
                    tmp = pf.tile([128, 512], F32, tag="tmp")
                    nc.vector.tensor_tensor(acc, xsb[:, 0, sl], pw_sb[:, n, 0], OP.mult)
                    for dc in range(1, KC):
                        nc.vector.tensor_tensor(tmp, xsb[:, dc, sl], pw_sb[:, n, dc],
                                                OP.mult)
                        nc.vector.tensor_tensor(acc, acc, tmp, OP.add)
                    r2 = pf.tile([128, 1], F32, tag="r2")
                    nc.vector.tensor_reduce(r2, acc, AX.X, OP.add)
                    o_ps = ppf.tile([1, 1], F32, tag="o")
                    nc.tensor.matmul(o_ps, ones32, r2,
                                     start=True, stop=True)
                    nc.vector.tensor_copy(ob_out[:, b * NCLS + n:b * NCLS + n + 1],
                                          o_ps)
            nc.vector.tensor_tensor(ob_out, ob_out, pb_sb, OP.add)
            nc.sync.dma_start(d_out, ob_out)


def _emit_attn(nc, tc, i, xsb, ft_sb, cm_sb, ident, ones_bf, d_qw, d_ow,
               d_qb, d_ob, d_wr, d_wi):
    from contextlib import ExitStack

    with ExitStack() as st:
        pwq = st.enter_context(tc.tile_pool(name=f"pwq{i}", bufs=1))
        pxbf = st.enter_context(tc.tile_pool(name=f"pxbf{i}", bufs=1))
        pq = st.enter_context(tc.tile_pool(name=f"pq{i}", bufs=5))
        psel = st.enter_context(tc.tile_pool(name=f"psel{i}", bufs=1))
        pwf = st.enter_context(tc.tile_pool(name=f"pwf{i}", bufs=2))
        pxt2 = st.enter_context(tc.tile_pool(name=f"pxt2{i}", bufs=2))
        pot = st.enter_context(tc.tile_pool(name=f"pot{i}", bufs=2))
        pmisc = st.enter_context(tc.tile_pool(name=f"pmisc{i}", bufs=1))

        qw_sb = pwq.tile([128, KC, D], BF16, tag="pw")
        nc.sync.dma_start(qw_sb, d_qw[i].rearrange("(c p) n -> p c n", p=128))
        qb_sb = pmisc.tile([1, D], BF16, tag="qb")
        nc.sync.dma_start(qb_sb, d_qb[i].rearrange("n -> () n"))
        ob_sb = pmisc.tile([128, KC], F32, tag="ob")
        nc.sync.dma_start(ob_sb, d_ob[i].rearrange("(c p) -> p c", p=128))

        x_bf = pxbf.tile([128, KC, T], BF16)
        for dc in range(KC):
            nc.vector.tensor_copy(x_bf[:, dc], xsb[:, dc])

        XSel = psel.tile([128, BL * D], BF16, tag="xsel")

        # ---- B+C: q-projection (token-major) then DFT ----
        with tc.tile_pool(name=f"ppq{i}", bufs=2, space="PSUM") as ppq, \
             tc.tile_pool(name=f"ppx{i}", bufs=2, space="PSUM") as ppx:
            for b in range(BL):
                q_ts = []
                for lc in range(4):
                    tt = b * 4 + lc
                    q_t = pq.tile([128, D], BF16)
                    for nh in range(2):
                        qp = ppq.tile([128, 512], F32)
                        for dc in range(KC):
                            nc.tensor.matmul(
                                qp,
                                x_bf[:, dc, tt * 128:(tt + 1) * 128],
                                qw_sb[:, dc, nh * 512:(nh + 1) * 512],
                                start=(dc == 0), stop=False,
                            )
                        # + qb via rank-1 ones trick
                        nc.tensor.matmul(
                            qp, ones_bf,
                            qb_sb[:, nh * 512:(nh + 1) * 512],
                            start=False, stop=True,
                        )
                        nc.vector.tensor_copy(q_t[:, nh * 512:(nh + 1) * 512], qp)
                    q_ts.append(q_t)
                for h in range(H):
                    psx = ppx.tile([128, 128], F32)
                    for lc in range(4):
                        nc.tensor.matmul(
                            psx,
                            q_ts[lc][:, h * 128:(h + 1) * 128],
                            ft_sb[:, lc],
                            start=(lc == 0), stop=(lc == 3),
                        )
                    nc.vector.tensor_copy(
                        XSel[:, b * D + h * 128:b * D + (h + 1) * 128], psx
                    )

        # ---- D: per-(h, m) complex mode mixing ----
        OSel = psel.tile([128, BL * D], BF16, tag="osel")
        xsel_v = XSel.rearrange("p (b h m) -> p h m b", b=BL, h=H)
        osel_v = OSel.rearrange("p (b h m) -> p h m b", b=BL, h=H)
        with tc.tile_pool(name=f"ppd{i}", bufs=3, space="PSUM") as ppd:
            for h in range(H):
                for half in range(4):
                    wr_sl = pwf.tile([128, 16, 128], BF16, tag="wr")
                    nc.sync.dma_start(wr_sl, d_wr[i, h, :, half * 16:(half + 1) * 16])
                    wi_sl = pwf.tile([128, 16, 128], BF16, tag="wi")
                    nc.sync.dma_start(wi_sl, d_wi[i, h, :, half * 16:(half + 1) * 16])
                    for mm in range(16):
                        m = half * 16 + mm
                        rhs = xsel_v[:, h, m::64, :]            # [128, 2, 4]
                        p1 = ppd.tile([128, 2, 4], F32, tag="p1")
                        p2 = ppd.tile([128, 2, 4], F32, tag="p2")
                        nc.tensor.matmul(p1, wr_sl[:, mm], rhs, start=True, stop=True)
                        nc.tensor.matmul(p2, wi_sl[:, mm], rhs, start=True, stop=True)
                        # DVE can read at most one PSUM operand: stage p2 in SBUF
                        s2 = pot.tile([128, 2, 4], F32, tag="s2")
                        nc.vector.tensor_copy(s2, p2)
                        nc.vector.tensor_tensor(osel_v[:, h, m, :], p1[:, 0],
                                                s2[:, 1], OP.subtract)
                        nc.vector.tensor_tensor(osel_v[:, h, m + 64, :], s2[:, 0],
                                                p1[:, 1], OP.add)

        # ---- E+F+G: transpose, iDFT, out-projection (+residual) ----
        ow_sb = pwq.tile([128, KC, D], BF16, tag="pw")
        nc.sync.dma_start(ow_sb, d_ow[i].rearrange("(c p) n -> p c n", p=128))
        with tc.tile_pool(name=f"ppe{i}", bufs=2, space="PSUM") as ppe, \
             tc.tile_pool(name=f"ppg{i}", bufs=2, space="PSUM") as ppg:
            for b in range(BL):
                xt2 = pxt2.tile([128, 4, D], BF16)
                for h in range(H):
                    tp = ppe.tile([128, 128], BF16, tag="tp")
                    nc.tensor.transpose(
                        tp, OSel[:, b * D + h * 128:b * D + (h + 1) * 128], ident
                    )
                    ot = pot.tile([128, 128], BF16)
                    nc.vector.tensor_copy(ot, tp)
                    fp = ppe.tile([128, 4, 128], F32, tag="fp")
                    for c in range(4):
                        nc.tensor.matmul(fp[:, c],
                                         cm_sb[:, c * 128:(c + 1) * 128], ot,
                                         start=True, stop=True)
                    nc.vector.tensor_copy(xt2[:, :, h * 128:(h + 1) * 128], fp)
                xt2v = xt2.rearrange("p c (h e) -> p c h e", h=H)
                for ncc in range(KC):
                    gp = ppg.tile([128, 512], F32)
                    k = 0
                    for s in range(2):
                        for c in range(4):
                            dpc = s * 4 + c
                            nc.tensor.matmul(
                                gp,
                                ow_sb[:, dpc, ncc * 128:(ncc + 1) * 128],
                                xt2v[:, c, :, s::2],
                                start=(k == 0), stop=(k == 7),
                            )
                            k += 1
                    xv = xsb[:, ncc, b * 512:(b + 1) * 512]
                    nc.vector.scalar_tensor_tensor(
                        xv, gp, ob_sb[:, ncc:ncc + 1], xv, OP.add, OP.add
                    )


def _emit_decomp(nc, tc, tag, xsb):
    with tc.tile_pool(name=f"pdc{tag}", bufs=2) as pd:
        for dc in range(KC):
            for b in range(BL):
                xs = xsb[:, dc, b * 512:(b + 1) * 512]
                xpad = pd.tile([128, 536], F32, tag="xpad")
                nc.gpsimd.tensor_copy(xpad[:, 12:524], xs)
                nc.gpsimd.tensor_copy(xpad[:, 0:12],
                                      xs[:, 0:1].to_broadcast([128, 12]))
                nc.gpsimd.tensor_copy(xpad[:, 524:536],
                                      xs[:, 511:512].to_broadcast([128, 12]))
                cb = pd.tile([128, 537], F32, tag="cb")
                nc.gpsimd.memset(cb[:, 0:1], 0.0)
                nc.vector.tensor_tensor_scan(cb[:, 1:537], xpad, xpad, 0.0,
                                             OP.add, OP.bypass)
                mn = pd.tile([128, 512], F32, tag="mn")
                nc.vector.tensor_tensor(mn, cb[:, 25:537], cb[:, 0:512], OP.subtract)
                # x -= mn/MA  (window-sum scaled to the moving average)
                nc.vector.scalar_tensor_tensor(xs, mn, -1.0 / MA, xs,
                                               OP.mult, OP.add)


def _emit_ffn(nc, tc, i, xsb, d_c1, d_c2):
    from contextlib import ExitStack

    def f32r(ap):
        return ap.bitcast(F32R)

    with ExitStack() as st:
        ph = st.enter_context(tc.tile_pool(name=f"ph{i}", bufs=1))
        pc1 = st.enter_context(tc.tile_pool(name=f"pc1{i}", bufs=3))
        pc2 = st.enter_context(tc.tile_pool(name=f"pc2{i}", bufs=3))
        pp1 = st.enter_context(tc.tile_pool(name=f"pp1{i}", bufs=2, space="PSUM"))
        pp2 = st.enter_context(tc.tile_pool(name=f"pp2{i}", bufs=1, space="PSUM"))
        for b in range(BL):
            sl = slice(b * 512, (b + 1) * 512)
            hs = ph.tile([128, FC, 512], F32R)
            for fc in range(FC):
                c1s = pc1.tile([128, KC, 128], F32R)
                nc.sync.dma_start(c1s, d_c1[i, fc])
                hp = pp1.tile([128, 512], F32)
                for dc in range(KC):
                    nc.tensor.matmul(hp, c1s[:, dc], xsb[:, dc, sl],
                                     start=(dc == 0), stop=(dc == KC - 1))
                nc.scalar.activation(hs[:, fc], hp, AF.Gelu)
            for halfn in range(2):
                gps = [pp2.tile([128, 512], F32, tag=f"y{j}", name=f"y{j}")
                       for j in range(4)]
                for fc in range(FC):
                    c2s = pc2.tile([128, 512], F32R)
                    nc.sync.dma_start(
                        c2s,
                        d_c2[i, fc * 128:(fc + 1) * 128,
                             halfn * 512:(halfn + 1) * 512],
                    )
                    for j in range(4):
                        nc.tensor.matmul(
                            gps[j], c2s[:, j * 128:(j + 1) * 128],
                            hs[:, fc],
                            start=(fc == 0), stop=(fc == FC - 1),
                        )
                for j in range(4):
                    dcn = halfn * 4 + j
                    xv = xsb[:, dcn, sl]
                    nc.vector.tensor_tensor(xv, xv, gps[j], OP.add)


# ---------------------------------------------------------------- entry point

_CACHE = {}


def kernel(**inputs) -> np.ndarray:
    if "nc" not in _CACHE:
        _CACHE["nc"] = build_nc()
    nc = _CACHE["nc"]
    in_maps = _host_prep(inputs)
    res = run_bass_kernel_spmd(nc, in_maps, core_ids=list(range(NCORES)))
    _CACHE["last_results"] = res
    outs = [r["out"].reshape(BL, NCLS) for r in res.results]
    return np.concatenate(outs, axis=0).astype(np.float32)
